# revision 28
# baseline (speedup 1.0000x reference)
"""AttentionNCF Trainium2 kernel (8-core SPMD, data-parallel over batch).

Math: reference computes
    scores[b,i] = cand[b]@w_c + rated[i]@w_r + b_att
    attn = softmax(where(user==0, -inf, scores), axis=i)
    user_est = (attn*user) @ rated ; then item/user towers + MLP.
Scores are rank-1 separable (a_b + r_i), so the per-row term a_b and b_att
cancel in the row softmax.  With v_i = exp(r_i):
    (attn*user)[b,i] = v_i * user[b,i] / s_b,   s_b = sum_i v_i * [user[b,i]!=0]
so attention is: wt = user * v (elementwise, v broadcast over b),
user_est[b,:] = (wt @ rated)[b,:] / s_b.  No (B,I) softmax passes needed.
All hidden-layer biases are jnp.zeros by construction -> omitted.

Design (v7; v2 baseline was 67-69us):
- Scores on the PE from a d-major fp8 copy of rated: r chunk c lands
  directly i-major as a (128,1) PSUM column via 4 accumulated matmuls
  (lhsT = ratedT d-chunk (128d x 128i) fp8, rhs = one wr column).  wr is
  pre-scaled by 64 so fp8 stays in range; the exp activation descales for
  free (exp(r'/64)).  This removes the entire score-reduce load (~40us
  combined) from DVE+ACT, which were the phase bottleneck in v2-v6.
- rated ships fp8 twice (i-major for the est matmul, d-major for scores):
  2+2MB, same DMA total as one bf16 copy.  est/s error from fp8 is
  ~0.03% (random per-element noise averaged over ~2048 attention terms,
  and est and s share the same quantized v).
- Single sync-queue DMA ordered by first use: wrt, then per-batch d-major
  slices interleaved with graduated rated/user groups, then per-layer
  weight packs.  The whole back half of the kernel streams at DMA rate.
- wt = ut*v batched per score-batch as one 3D DVE tensor_tensor with a
  stride-0 broadcast v operand, output fp8 (matches the est rhs dtype).
- s-matmul: lhsT = v column (1-col LDWEIGHTS), rhs = ind slice, into a
  (1,BS) PSUM row; ind is computed per user super-group (3 DVE ops).
- 1/s fuses into the est PSUM->SBUF copy (batch-major there), split
  DVE/ACT halves; reciprocal chain (fp16 transpose of the s row) runs
  concurrently with m1a matmuls.
- m_w1 splits: the item half accumulates into a held PSUM group right
  after the item tower (mid/late attention); only the user half is in
  the serial tail.  Tail relu copies: last block split across DVE+ACT.
- Towers fp16 weights/activations with fp32 PSUM.
"""

from contextlib import ExitStack

import ml_dtypes
import numpy as np

import concourse.bass as bass
import concourse.mybir as mybir
import concourse.tile as tile
from concourse import bacc
from concourse.bass_utils import run_bass_kernel_spmd
from concourse.masks import make_identity

B, I, D = 1024, 4096, 512
IE, UE = 256, 512
D1, D2, D3, D4 = 1024, 512, 256, 128
NCORES = 8
BS = B // NCORES   # 128 batch rows per core
NI = I // 128      # 32 i-chunks
ND = D // 128      # 4 d-chunks
WR_SCALE = 64.0    # wr pre-scale so fp8 e4m3 holds it; exp descales
RG_SIZES = [1, 1, 2, 4, 4, 4, 4, 6, 6]   # rated-group chunk counts
UG_SIZES = [(0, 8), (8, 8), (16, 16)]    # user super-groups (start, len)
BATCHES = [(0, 8), (8, 8), (16, 8), (24, 4), (28, 4)]

f32 = mybir.dt.float32
f16 = mybir.dt.float16
bf16 = mybir.dt.bfloat16
fp8 = mybir.dt.float8e4
AF = mybir.ActivationFunctionType
OP = mybir.AluOpType

LAYERS = {
    "ie_w1": (D, 2 * IE), "ie_w2": (2 * IE, IE),
    "ue_w1": (D, 2 * UE), "ue_w2": (2 * UE, UE),
    "m_w1a": (IE, D1), "m_w1b": (UE, D1),
    "m_w2": (D1, D2), "m_w3": (D2, D3), "m_w4": (D3, D4),
}
PACKS = {
    "cie1": ["candT", "ie_w1"],
    "cie2": ["ie_w2", "m_w1a"],
    "ue1": ["ue_w1"],
    "ue2": ["ue_w2"],
    "m1b": ["m_w1b"],
    "m2": ["m_w2"],
    "m3w45": ["m_w3", "m_w4", "w5"],
}
PACK_SHAPES = dict(LAYERS, candT=(D, BS), w5=(D4, 1))


def _pack_offsets():
    offs = {}
    for pack, names in PACKS.items():
        off = 0
        for n in names:
            K, F = PACK_SHAPES[n]
            offs[n] = (pack, off, K, F)
            off += (K // 128) * F
        offs[pack + "__total"] = off
    return offs


POFF = _pack_offsets()


def build_nc():
    nc = bacc.Bacc(
        "TRN2", target_bir_lowering=False, debug=False, num_devices=NCORES
    )

    wrt_ap = nc.dram_tensor("wrt", [128, ND], fp8, kind="ExternalInput").ap()
    dmaj_ap = [
        nc.dram_tensor(f"dmaj{bi}", [128, ND, 128 * bn], fp8,
                       kind="ExternalInput").ap()
        for bi, (b0, bn) in enumerate(BATCHES)
    ]
    rg_ap = [
        nc.dram_tensor(f"rg{g}", [128, n, D], fp8, kind="ExternalInput").ap()
        for g, n in enumerate(RG_SIZES)
    ]
    ug_ap = [
        nc.dram_tensor(f"ug{s}", [128, m, BS], bf16, kind="ExternalInput").ap()
        for s, (s0, m) in enumerate(UG_SIZES)
    ]
    pk_ap = {}
    for pack in PACKS:
        pk_ap[pack] = nc.dram_tensor(
            pack, [128, POFF[pack + "__total"]], f16, kind="ExternalInput"
        ).ap()
    out = nc.dram_tensor("out", [BS, 1], f32, kind="ExternalOutput").ap()

    with tile.TileContext(nc) as tc, ExitStack() as ctx:
        pool = ctx.enter_context(tc.tile_pool(name="main", bufs=1))
        wt_pool = ctx.enter_context(tc.tile_pool(name="wt", bufs=3))
        psum_att = ctx.enter_context(tc.tile_pool(name="psA", bufs=1, space="PSUM"))
        psum_s = ctx.enter_context(tc.tile_pool(name="psS", bufs=1, space="PSUM"))
        psum_rc = ctx.enter_context(tc.tile_pool(name="psRC", bufs=2, space="PSUM"))
        psum_layer = ctx.enter_context(tc.tile_pool(name="psL", bufs=2, space="PSUM"))
        psum_m1 = ctx.enter_context(tc.tile_pool(name="psM1", bufs=1, space="PSUM"))

        identity = pool.tile([128, 128], f16)
        make_identity(nc, identity[:])

        # ---- DMAs: one sync HWDGE queue ordered by first use.
        wrt = pool.tile([128, ND], fp8)
        nc.sync.dma_start(wrt[:], wrt_ap[:, :])

        dmaj_t = []
        rg_t = []
        ug_t = []
        pk_tiles = {}

        def dma_dmaj(bi):
            b0, bn = BATCHES[bi]
            t = pool.tile([128, ND, 128 * bn], fp8, tag=f"dmaj{bi}")
            nc.sync.dma_start(t[:], dmaj_ap[bi][:, :, :])
            dmaj_t.append(t)

        def dma_rg(g):
            t = pool.tile([128, RG_SIZES[g], D], fp8, tag=f"rg{g}")
            nc.sync.dma_start(t[:], rg_ap[g][:, :, :])
            rg_t.append(t)

        def dma_ug(s):
            t = pool.tile([128, UG_SIZES[s][1], BS], bf16, tag=f"ug{s}")
            nc.sync.dma_start(t[:], ug_ap[s][:, :, :])
            ug_t.append(t)

        def dma_pack(pack):
            t = pool.tile([128, POFF[pack + "__total"]], f16, tag=pack)
            nc.sync.dma_start(t[:], pk_ap[pack][:, :])
            pk_tiles[pack] = t

        dma_dmaj(0)
        dma_dmaj(1)
        for g in (0, 1, 2, 3):
            dma_rg(g)
        dma_ug(0)
        dma_dmaj(2)
        dma_rg(4)
        dma_rg(5)
        dma_ug(1)
        dma_dmaj(3)
        dma_dmaj(4)
        for g in (6, 7, 8):
            dma_rg(g)
        dma_ug(2)
        for pk in ("cie1", "cie2", "ue1", "ue2", "m1b", "m2", "m3w45"):
            dma_pack(pk)

        def wslice(name, k, f0, fn=128):
            pack, off, K, F = POFF[name]
            base = off + k * F + f0
            return pk_tiles[pack][:, base:base + fn]

        def rated_c(c):
            """(128, D) fp8 i-major rated chunk c."""
            g0 = 0
            for g, n in enumerate(RG_SIZES):
                if g0 <= c < g0 + n:
                    return rg_t[g][:, c - g0, :]
                g0 += n
            raise AssertionError

        def ut_view(c):
            """(user super tile, super start, super len) for chunk c."""
            for s, (s0, m) in enumerate(UG_SIZES):
                if s0 <= c < s0 + m:
                    return ug_t[s], s0, m
            raise AssertionError

        v_all = pool.tile([128, NI], f32)
        v_bf = pool.tile([128, NI], bf16)

        # ---- Weight-stationary tower layer helper ----
        def wlayer(xT_chunks, wname, relu_eng="vector", scale=None,
                   pipelined=False):
            """hT = relu(W.T @ x) with x given as K-major 128-chunks.
            pipelined: split the layer's final (gating) 512-block relu
            across DVE+ACT halves so the next layer's wait shrinks.
            scale: per-partition (BS,1) fp32 AP multiplied in before relu.
            Returns list of (128, BS) chunk APs of the output."""
            K, F = PACK_SHAPES[wname]
            nk = K // 128
            assert len(xT_chunks) == nk
            hT = pool.tile([128, F], f16, tag=f"h_{wname}")
            nblk = (F + 511) // 512
            for bi, f0 in enumerate(range(0, F, 512)):
                fn = min(512, F - f0)
                ps = psum_layer.tile([BS, fn], f32, tag="psL")
                for fs in range(0, fn, 128):
                    for k in range(nk):
                        nc.tensor.matmul(
                            ps[:, fs:fs + 128],
                            lhsT=wslice(wname, k, f0 + fs),
                            rhs=xT_chunks[k],
                            start=(k == 0), stop=(k == nk - 1),
                        )
                last = (bi == nblk - 1)
                if pipelined and last and fn >= 256:
                    h = fn // 2
                    _relu_v(hT[:, f0:f0 + h], ps[:, :h], scale)
                    _relu_a(hT[:, f0 + h:f0 + fn], ps[:, h:], scale)
                else:
                    dst = hT[:, f0:f0 + fn]
                    if (relu_eng == "vector") or (pipelined and bi % 2 == 0):
                        _relu_v(dst, ps[:], scale)
                    else:
                        _relu_a(dst, ps[:], scale)
            return [hT[:, j * 128:(j + 1) * 128] for j in range(F // 128)]

        def _relu_v(dst, src, scale):
            if scale is None:
                nc.vector.tensor_scalar_max(dst, src, 0.0)
            else:
                nc.vector.tensor_scalar(dst, src, scale, 0.0, OP.mult, OP.max)

        def _relu_a(dst, src, scale):
            if scale is None:
                nc.scalar.activation(dst, src, AF.Relu)
            else:
                nc.scalar.activation(dst, src, AF.Relu, scale=scale)

        candT_chunks = [wslice("candT", 0, j * 128) for j in range(D // 128)]
        item_out = {}

        def emit_h1():
            item_out["h1"] = wlayer(candT_chunks, "ie_w1", relu_eng="scalar")

        def emit_ie():
            item_out["ie"] = wlayer(item_out["h1"], "ie_w2", relu_eng="scalar")

        # m_w1 item half: held PSUM accumulation group (user half closes
        # it in the tail).
        m1_ps = []
        for f0 in (0, 512):
            m1_ps_blk = psum_m1.tile([BS, 512], f32, tag=f"m1_{f0}")
            m1_ps.append(m1_ps_blk)

        def emit_m1a(bi):
            f0 = (0, 512)[bi]
            for fs in range(0, 512, 128):
                for k in range(2):
                    nc.tensor.matmul(
                        m1_ps[bi][:, fs:fs + 128],
                        lhsT=wslice("m_w1a", k, f0 + fs),
                        rhs=item_out["ie"][k],
                        start=(fs == 0 and k == 0), stop=False,
                    )

        # ---- Attention.
        est_psum = psum_att.tile([BS, D], f32)
        s_row = psum_s.tile([1, BS], f32)
        ind_tiles = {}   # super start -> ind tile (128, m, BS) bf16

        def emit_scores(bi):
            b0, bn = BATCHES[bi]
            rps = psum_rc.tile([128, bn], f32, tag="rc")
            dm = dmaj_t[bi]
            for c in range(b0, b0 + bn):
                q = c - b0
                for dk in range(ND):
                    nc.tensor.matmul(
                        rps[:, q:q + 1],
                        lhsT=dm[:, dk, q * 128:(q + 1) * 128],
                        rhs=wrt[:, dk:dk + 1],
                        start=(q == 0 and dk == 0),
                        stop=(q == bn - 1 and dk == ND - 1),
                    )
            sl = slice(b0, b0 + bn)
            nc.scalar.activation(v_all[:, sl], rps[:, :], AF.Exp,
                                 scale=1.0 / WR_SCALE)
            nc.vector.tensor_copy(v_bf[:, sl], v_all[:, sl])

        def emit_attn(bi):
            b0, bn = BATCHES[bi]
            ut, s0, m = ut_view(b0)
            if s0 not in ind_tiles:
                ind = wt_pool.tile([128, m, BS], bf16, tag=f"ind{s0}")
                nc.vector.tensor_scalar(
                    ind[:, :, :], ut[:, :, :], 0.0, None, OP.is_gt
                )
                ind_tiles[s0] = ind
            j0 = b0 - s0
            wt = wt_pool.tile([128, bn, BS], fp8, tag="wt")
            nc.vector.tensor_tensor(
                wt[:, :, :], ut[:, j0:j0 + bn, :],
                v_all[:, b0:b0 + bn, None].broadcast_to([128, bn, BS]),
                OP.mult,
            )
            for c in range(b0, b0 + bn):
                q = c - b0
                nc.tensor.matmul(
                    est_psum[:], lhsT=wt[:, q, :], rhs=rated_c(c),
                    start=(c == 0), stop=(c == NI - 1),
                )
                nc.tensor.matmul(
                    s_row[:], lhsT=v_bf[:, c:c + 1],
                    rhs=ind_tiles[s0][:, j0 + q, :],
                    start=(c == 0), stop=(c == NI - 1),
                )

        emit_scores(0)
        for bi in range(1, len(BATCHES)):
            emit_scores(bi)
            emit_attn(bi - 1)
        emit_attn(len(BATCHES) - 1)
        emit_h1()
        emit_ie()

        # ---- s epilogue: s row -> fp16 -> transpose to (BS,1) -> +eps ->
        # reciprocal.  m1a matmuls fill the PE while the chain resolves.
        s_sb = pool.tile([1, BS], f16)
        nc.vector.tensor_copy(s_sb[:], s_row[:])
        s_col_ps = psum_layer.tile([BS, 1], f16, tag="psL")
        nc.tensor.transpose(s_col_ps[:], s_sb[:], identity[:1, :1])
        s_eps = pool.tile([BS, 1], f32)
        nc.vector.tensor_scalar_add(s_eps[:], s_col_ps[:], 1e-30)
        recip = pool.tile([BS, 1], f32)
        nc.vector.reciprocal(recip[:], s_eps[:])
        emit_m1a(0)

        # ---- est epilogue: 1/s fused into the PSUM->SBUF copy, split
        # across DVE/ACT halves, then transpose to K-major.
        est = pool.tile([BS, D], f16)
        nc.vector.tensor_scalar(
            est[:, :256], est_psum[:, :256], recip[:], None, OP.mult)
        nc.scalar.activation(
            est[:, 256:], est_psum[:, 256:], AF.Copy, scale=recip[:])
        tp = psum_layer.tile([128, D], f16, tag="psL")
        for j in range(4):
            nc.tensor.transpose(
                tp[:, j * 128:(j + 1) * 128],
                est[:, j * 128:(j + 1) * 128], identity[:],
            )
        estT = pool.tile([128, D], f16)
        nc.vector.tensor_copy(estT[:], tp[:])
        estT_chunks = [estT[:, j * 128:(j + 1) * 128] for j in range(4)]

        # ---- user tower + MLP tail.
        u1 = wlayer(estT_chunks, "ue_w1", pipelined=True)
        emit_m1a(1)
        u2 = wlayer(u1, "ue_w2", pipelined=True)

        # finish m_w1 (user half).
        m1h = pool.tile([128, D1], f16, tag="h_m1")
        for bi, f0 in enumerate((0, 512)):
            for fs in range(0, 512, 128):
                for k in range(4):
                    nc.tensor.matmul(
                        m1_ps[bi][:, fs:fs + 128],
                        lhsT=wslice("m_w1b", k, f0 + fs),
                        rhs=u2[k],
                        start=False, stop=(fs == 384 and k == 3),
                    )
            _relu_v(m1h[:, f0:f0 + 256], m1_ps[bi][:, :256], None)
            _relu_a(m1h[:, f0 + 256:f0 + 512], m1_ps[bi][:, 256:], None)
        m1 = [m1h[:, j * 128:(j + 1) * 128] for j in range(D1 // 128)]

        m2 = wlayer(m1, "m_w2", pipelined=True)
        m3 = wlayer(m2, "m_w3", pipelined=True)
        m4 = wlayer(m3, "m_w4", pipelined=True)
        out_ps = psum_layer.tile([BS, 1], f32, tag="psL")
        nc.tensor.matmul(
            out_ps[:], lhsT=m4[0], rhs=wslice("w5", 0, 0, fn=1),
            start=True, stop=True,
        )
        out_sb = pool.tile([BS, 1], f32)
        nc.vector.tensor_copy(out_sb[:], out_ps[:])
        nc.sync.dma_start(out[:, :], out_sb[:])

    nc.compile()
    return nc


_NC_CACHE = None


def get_nc():
    global _NC_CACHE
    if _NC_CACHE is None:
        _NC_CACHE = build_nc()
    return _NC_CACHE


def _shuffle(x, dtype):
    """(K, F) row-major -> (128, K/128, F) partition-major contiguous."""
    K, F = x.shape
    return np.ascontiguousarray(
        x.reshape(K // 128, 128, F).transpose(1, 0, 2).astype(dtype))


def make_in_maps(inputs):
    e4m3 = ml_dtypes.float8_e4m3
    cand = np.asarray(inputs["candidate_items"], np.float32)
    rated = np.asarray(inputs["rated_items"], np.float32)
    user = np.asarray(inputs["user_matrix"], np.float32)
    w_att = np.asarray(inputs["w_att"], np.float32)

    wr64 = (w_att[D:, 0] * WR_SCALE).reshape(ND, 128)
    wrt = np.ascontiguousarray(wr64.T.astype(e4m3))          # (128, ND)

    # d-major rated: (128, ND, I); per-batch column slices.
    dmajf = np.ascontiguousarray(
        rated.T.reshape(ND, 128, I).transpose(1, 0, 2).astype(e4m3))
    dmaj = {}
    for bi, (b0, bn) in enumerate(BATCHES):
        dmaj[f"dmaj{bi}"] = np.ascontiguousarray(
            dmajf[:, :, b0 * 128:(b0 + bn) * 128])

    rated_sh = _shuffle(rated, e4m3)                         # (128, NI, D)
    rg = {}
    c0 = 0
    for g, n in enumerate(RG_SIZES):
        rg[f"rg{g}"] = np.ascontiguousarray(rated_sh[:, c0:c0 + n])
        c0 += n

    def pack(pname, mats):
        parts = []
        for name in PACKS[pname]:
            parts.append(_shuffle(mats[name], np.float16).reshape(128, -1))
        return np.ascontiguousarray(np.concatenate(parts, axis=1))

    m_w1 = np.asarray(inputs["m_w1"], np.float32)
    shared_mats = {
        "ie_w1": np.asarray(inputs["ie_w1"], np.float32),
        "ie_w2": np.asarray(inputs["ie_w2"], np.float32),
        "ue_w1": np.asarray(inputs["ue_w1"], np.float32),
        "ue_w2": np.asarray(inputs["ue_w2"], np.float32),
        "m_w1a": np.ascontiguousarray(m_w1[:IE]),
        "m_w1b": np.ascontiguousarray(m_w1[IE:]),
        "m_w2": np.asarray(inputs["m_w2"], np.float32),
        "m_w3": np.asarray(inputs["m_w3"], np.float32),
        "m_w4": np.asarray(inputs["m_w4"], np.float32),
        "w5": np.asarray(inputs["m_w5"], np.float32),
    }
    shared = {
        "wrt": wrt, **dmaj, **rg,
        "cie2": pack("cie2", shared_mats),
        "ue1": pack("ue1", shared_mats),
        "ue2": pack("ue2", shared_mats),
        "m1b": pack("m1b", shared_mats),
        "m2": pack("m2", shared_mats),
        "m3w45": pack("m3w45", shared_mats),
    }

    in_maps = []
    for core in range(NCORES):
        sl = slice(core * BS, (core + 1) * BS)
        ut_sh = _shuffle(np.ascontiguousarray(user[sl].T),
                         ml_dtypes.bfloat16)                 # (128, NI, BS)
        ug = {}
        for s, (s0, m) in enumerate(UG_SIZES):
            ug[f"ug{s}"] = np.ascontiguousarray(ut_sh[:, s0:s0 + m])
        mats = dict(shared_mats)
        mats["candT"] = np.ascontiguousarray(cand[sl].T)
        in_maps.append({
            "cie1": pack("cie1", mats),
            **ug, **shared,
        })
    return in_maps


def kernel(**inputs) -> np.ndarray:
    nc = get_nc()
    res = run_bass_kernel_spmd(nc, make_in_maps(inputs), list(range(NCORES)))
    return np.concatenate([r["out"] for r in res.results], axis=0)
